# revision 31
# baseline (speedup 1.0000x reference)
"""Trainium2 Bass kernel: 12-head self-attention block (B=2, N=4096, C=768).

Sharding: token-parallel over the 8192 (batch, token) rows. Core c (0..7)
handles batch c//4, query rows [(c%4)*1024, (c%4+1)*1024). Every core
redundantly computes K/V for its whole batch (zero collectives); the host
rotates each core's token order so its own 1024 query tokens come first
(attention is permutation-invariant over keys).

The kernel is exp-bound-ish: softmax exponentiates 50M scores per core and
the ACT engine (1.2 GHz, 128 lanes) needs ~430us; the PE needs ~580us for
all matmuls at the measured warm issue rate, so wall time is close to the
sum of phase A (PE only) and phase C (ACT-bound with PE just under it).
Structure notes:

  * All staging in SBUF (no DRAM scratch roundtrips), bf16 everywhere.
    fp8 V was tested and rejected: attention averages v into small
    outputs, so fp8's ~3.6% mantissa noise survives relatively
    undiminished (2.1e-2 rel err, over the 2e-2 gate).
  * Dense back-to-back matmul streaming keeps the PE HAM clock gate at
    2.4 GHz. Idle gaps or slow DVE ops between matmuls re-throttle it to
    1.2 GHz - the naive DRAM-scratch version measured 945us throttled
    because ~50k tiny DMA descriptors (128B lines and 2-byte ones-column
    writes) starved the PE, and every matmul ran cold-isolated.
  * Scores put the two heads of a pair at PE row tiles T0/T8 (partition
    bases 0/64, K=64) which stream concurrently.
  * v_aug carries a ones column so the AV matmul emits softmax
    denominators for free (row 64); reciprocal_approx_fast (input
    relocated to partition 0 - the custom DVE op's NR constants live
    there) feeds a gpsimd broadcast and one DVE multiply per tile.

Device pipeline per core:
  phase A (per 1024-token quarter): qT/kT[col,t] = Wqkv[:, :1536].T @ x^T
           (q only for quarter 0 -> qT_sb; k -> kT_sb), v[t,(h,d)] =
           x @ Wqkv[:, 1536:] -> v_sb[t, kc, h, 0:64], ones at col 64.
  phase C: per head-pair hp, key-chunk kc: scoresT[key,q] = kT_h.T @ qT_h,
           eT = exp(SCALE*sT) on ACT, av[0:65] += v_aug.T @ eT (two
           iterations behind its exp; et pool holds the slack).
  phase D: out[t,c] = tokensT.T @ Wproj + bproj.
"""

import sys

import numpy as np

try:
    import concourse  # noqa: F401
except ImportError:  # pragma: no cover
    sys.path.insert(0, "/opt/trn_rl_repo")

import ml_dtypes

import concourse.bass as bass  # noqa: F401
import concourse.mybir as mybir
import concourse.tile as tile
from concourse import bacc
from concourse.bass_utils import run_bass_kernel_spmd

B, N, C = 2, 4096, 768
H, D = 12, 64
NT = 1024  # query tokens per core
SCALE = float(D) ** -0.5
NCORES = 8
KC = N // 128  # 32 key chunks per batch
VW = D + 1  # v_aug row width per head: [v(64), ones]

F32 = mybir.dt.float32
BF16 = mybir.dt.bfloat16
EXP = mybir.ActivationFunctionType.Exp
MUL = mybir.AluOpType.mult
ADD = mybir.AluOpType.add


def build_graph():
    nc = bacc.Bacc(
        "TRN2", target_bir_lowering=False, debug=False, num_devices=NCORES
    )

    xT_e = nc.declare_dram_parameter("xT", [C, N], BF16, isOutput=False)
    wqkv_e = nc.declare_dram_parameter("Wqkv", [C, 3 * C], BF16, isOutput=False)
    wproj_e = nc.declare_dram_parameter("Wproj", [C, C], BF16, isOutput=False)
    bproj_e = nc.declare_dram_parameter("bproj", [1, C], F32, isOutput=False)
    out_e = nc.declare_dram_parameter("out", [NT, C], F32, isOutput=True)

    with tile.TileContext(nc) as tc:
        _build_body(nc, tc, xT_e, wqkv_e, wproj_e, bproj_e, out_e)
    nc.finalize()
    return nc


def _build_body(nc, tc, xT_e, wqkv_e, wproj_e, bproj_e, out_e):
    with tc.tile_pool(name="persist", bufs=1) as persist:
        # ---- persistent SBUF ----
        qT_sb = persist.tile([128, 6, NT], BF16, tag="qT")
        kT_sb = persist.tile([128, 6, N], BF16, tag="kT")
        # v_aug[token_part, key_chunk, head, 0:64]=v, [.,.,.,64]=1.0
        v_sb = persist.tile([128, KC, H, VW], BF16, tag="v")
        tokT = [
            persist.tile([128, NT], BF16, tag=f"tokT{i}", name=f"tokT{i}")
            for i in range(6)
        ]
        # phase D weights, prefetched at graph start
        wproj_sb = persist.tile([128, 6, C], BF16, tag="wproj")
        bproj_sb = persist.tile([1, C], F32, tag="bproj")
        bproj_bc = persist.tile([128, C], F32, tag="bproj_bc")

        # full-tile contiguous memset (single-dim AP); v writes then
        # overwrite cols 0:63 of each head slot, leaving col 64 == 1.0
        nc.vector.memset(v_sb[:], 1.0)
        nc.sync.dma_start(bproj_sb[:], bproj_e[:])
        nc.gpsimd.partition_broadcast(bproj_bc[:], bproj_sb[:])
        for cc in range(6):
            nc.sync.dma_start(
                wproj_sb[:, cc, :], wproj_e[cc * 128 : (cc + 1) * 128, :]
            )

        # ================= phase A: qkv projection =================
        with (
            tc.tile_pool(name="pa_w", bufs=1) as paw,
            tc.tile_pool(name="pa_x", bufs=2) as pax,
            tc.tile_pool(name="pa_psum", bufs=2, space="PSUM") as pap,
        ):
            wqkv_sb = paw.tile([128, 6, 3 * C], BF16, tag="wqkv")
            for kc in range(6):
                nc.sync.dma_start(
                    wqkv_sb[:, kc, :], wqkv_e[kc * 128 : (kc + 1) * 128, :]
                )

            for tq in range(4):  # 1024-token quarters
                tq0 = tq * NT
                xq = pax.tile([128, 6, NT], BF16, tag="xq")
                for kc in range(6):
                    nc.sync.dma_start(
                        xq[:, kc, :],
                        xT_e[kc * 128 : (kc + 1) * 128, tq0 : tq0 + NT],
                    )

                # q (quarter 0 only) + k columns, transposed orientation.
                # Both token halves share each weight chunk (one LDW, 2 MMs).
                ccs = range(12) if tq == 0 else range(6, 12)
                for cc in ccs:
                    pj = pap.tile([128, 1024], F32, tag="pj")
                    for kc in range(6):
                        for th in range(2):
                            nc.tensor.matmul(
                                pj[:, th * 512 : (th + 1) * 512],
                                wqkv_sb[:, kc, cc * 128 : (cc + 1) * 128],
                                xq[:, kc, th * 512 : (th + 1) * 512],
                                start=(kc == 0),
                                stop=(kc == 5),
                            )
                    if cc < 6:
                        nc.vector.tensor_copy(qT_sb[:, cc, :], pj[:])
                    else:
                        nc.vector.tensor_copy(
                            kT_sb[:, cc - 6, tq0 : tq0 + NT], pj[:]
                        )

                # v columns (token-major); one xq LDW feeds both col halves
                for tcn in range(8):
                    kcn = tq * 8 + tcn  # global 128-token (=key) chunk
                    pj = pap.tile([128, 768], F32, tag="pjv")
                    for kc in range(6):
                        for c0, c1 in ((0, 512), (512, 768)):
                            nc.tensor.matmul(
                                pj[:, c0:c1],
                                xq[:, kc, tcn * 128 : (tcn + 1) * 128],
                                wqkv_sb[:, kc, 2 * C + c0 : 2 * C + c1],
                                start=(kc == 0),
                                stop=(kc == 5),
                            )
                    nc.vector.tensor_copy(
                        v_sb[:, kcn, :, 0:D],
                        pj[:].rearrange("p (h d) -> p h d", d=D),
                    )

        # ================= phase C: attention =================
        # Per kc: 4 K=64 score MMs (heads at PE row tiles T0/T8 stream
        # concurrently), 2 exps on ACT, 4 K=128 AV MMs two iterations
        # behind (et pool holds the slack so PE never waits on ACT).
        # PSUM: sc 2x[128,1024] (2 banks each) + av 4x[65,512] = 8 banks.
        with (
            tc.tile_pool(name="et_pool", bufs=12) as etp,
            tc.tile_pool(name="sc_pool", bufs=2, space="PSUM") as scp,
            tc.tile_pool(name="av_pool", bufs=4, space="PSUM") as avp,
            tc.tile_pool(name="small", bufs=4) as smp,
        ):
            for hp in range(6):
                avs = []
                for i in range(4):
                    avt = avp.tile([65, 512], F32, tag="av", name=f"av_{hp}_{i}")
                    avs.append(avt)

                # AV for kc runs 2 iterations behind its exp
                def do_av(kc, e0, e1):
                    for hd, et in ((0, e0), (1, e1)):
                        for qh in range(2):
                            jsl = slice(qh * 512, (qh + 1) * 512)
                            nc.tensor.matmul(
                                avs[2 * hd + qh][:],
                                v_sb[:, kc, 2 * hp + hd, :],
                                et[:, jsl],
                                start=(kc == 0),
                                stop=(kc == KC - 1),
                            )

                pend = []
                for kc in range(KC):
                    ksl = slice(kc * 128, (kc + 1) * 128)
                    sc0 = scp.tile([128, 1024], F32, tag="sc")
                    sc1 = scp.tile([128, 1024], F32, tag="sc")
                    for qh in range(2):
                        qsl = slice(qh * 512, (qh + 1) * 512)
                        nc.tensor.matmul(
                            sc0[:, qsl],
                            kT_sb[0:64, hp, ksl],
                            qT_sb[0:64, hp, qsl],
                            start=True,
                            stop=True,
                        )
                    for qh in range(2):
                        qsl = slice(qh * 512, (qh + 1) * 512)
                        nc.tensor.matmul(
                            sc1[:, qsl],
                            kT_sb[64:128, hp, ksl],
                            qT_sb[64:128, hp, qsl],
                            start=True,
                            stop=True,
                        )
                    e0 = etp.tile([128, 1024], BF16, tag="et")
                    e1 = etp.tile([128, 1024], BF16, tag="et")
                    nc.scalar.activation(e0[:], sc0[:], EXP, scale=SCALE)
                    nc.scalar.activation(e1[:], sc1[:], EXP, scale=SCALE)
                    pend.append((kc, e0, e1))
                    if len(pend) > 2:
                        do_av(*pend.pop(0))
                for p in pend:
                    do_av(*p)

                for i, av in enumerate(avs):
                    hd, qh = i // 2, i % 2
                    qsl = slice(qh * 512, (qh + 1) * 512)
                    # one copy releases the PSUM bank; the recip chain
                    # then runs off the PE critical path
                    av_sb = smp.tile([65, 512], F32, tag="av_sb")
                    nc.vector.tensor_copy(av_sb[:], av[:])
                    # relocate denominator row to partition 0 (1-partition
                    # DVE copies cross quadrants; the custom approx-recip op
                    # needs all operands co-resident at partition 0)
                    den = smp.tile([1, 512], F32, tag="den")
                    nc.vector.tensor_copy(den[:], av_sb[64:65, :])
                    rec = smp.tile([1, 512], F32, tag="rec")
                    nc.vector.reciprocal_approx_fast(rec[:], den[:])
                    bc = smp.tile([64, 512], F32, tag="bc")
                    nc.gpsimd.partition_broadcast(bc[:], rec[:])
                    if hd == 0:
                        nc.vector.tensor_tensor(
                            out=tokT[hp][0:64, qsl],
                            in0=av_sb[0:64, :],
                            in1=bc[:],
                            op=MUL,
                        )
                    else:
                        tmp = smp.tile([64, 512], BF16, tag="tmp")
                        nc.vector.tensor_tensor(
                            out=tmp[:], in0=av_sb[0:64, :], in1=bc[:], op=MUL
                        )
                        # partition-shifting copy (base 0 -> 64) via DMA
                        nc.sync.dma_start(tokT[hp][64:128, qsl], tmp[:])

        # ================= phase D: output projection =================
        with (
            tc.tile_pool(name="pd_psum", bufs=4, space="PSUM") as pdp,
            tc.tile_pool(name="pd_sbuf", bufs=4) as pds,
        ):
            for tcn in range(8):
                pj = pdp.tile([128, 768], F32, tag="pd")
                for cc in range(6):
                    for c0, c1 in ((0, 512), (512, 768)):
                        nc.tensor.matmul(
                            pj[:, c0:c1],
                            tokT[cc][:, tcn * 128 : (tcn + 1) * 128],
                            wproj_sb[:, cc, c0:c1],
                            start=(cc == 0),
                            stop=(cc == 5),
                        )
                ot = pds.tile([128, 768], F32, tag="ot")
                nc.vector.tensor_tensor(
                    out=ot[:], in0=pj[:], in1=bproj_bc[:], op=ADD
                )
                nc.sync.dma_start(out_e[tcn * 128 : (tcn + 1) * 128, :], ot[:])


_CACHE = {}


def _get_graph():
    if "nc" not in _CACHE:
        _CACHE["nc"] = build_graph()
    return _CACHE["nc"]


def make_in_maps(x, W_qkv, W_proj, b_proj):
    x = np.asarray(x, dtype=np.float32)
    W_qkv = np.asarray(W_qkv, dtype=np.float32).astype(ml_dtypes.bfloat16)
    W_proj = np.asarray(W_proj, dtype=np.float32).astype(ml_dtypes.bfloat16)
    b_proj = np.asarray(b_proj, dtype=np.float32).reshape(1, C)
    W_qkv = np.ascontiguousarray(W_qkv)
    W_proj = np.ascontiguousarray(W_proj)
    in_maps = []
    for c in range(NCORES):
        bb, r0 = c // 4, (c % 4) * NT
        idx = np.r_[r0 : r0 + NT, 0:r0, r0 + NT : N]
        xT = np.ascontiguousarray(
            x[bb][idx].T.astype(ml_dtypes.bfloat16)
        )  # own tokens first
        in_maps.append(
            {
                "xT": xT,
                "Wqkv": W_qkv,
                "Wproj": W_proj,
                "bproj": b_proj,
            }
        )
    return in_maps


def run(x, W_qkv, W_proj, b_proj, trace=False):
    nc = _get_graph()
    in_maps = make_in_maps(x, W_qkv, W_proj, b_proj)
    res = run_bass_kernel_spmd(
        nc, in_maps, core_ids=list(range(NCORES)), trace=trace
    )
    out = np.zeros((B, N, C), dtype=np.float32)
    for c in range(NCORES):
        bb, r0 = c // 4, (c % 4) * NT
        out[bb, r0 : r0 + NT, :] = res.results[c]["out"]
    return out, res


def kernel(x, W_qkv, W_proj, b_proj):
    out, _ = run(x, W_qkv, W_proj, b_proj, trace=False)
    return out
